# revision 8
# baseline (speedup 1.0000x reference)
"""BirthDeathAttention kernel for 8 Trainium2 NeuronCores.

Math note: in the reference, both `persistence_bias` ([1,H,1,1]) and
`importance_weights[:, None, :, None] * 0.1` ([B,1,N,1]) are constant along
the softmax (key) axis, so they cancel exactly inside the softmax.  The
module is therefore plain multi-head attention + output projection.

Sharding (per the tensor-parallel hint): core = (batch b, head-group g),
b in {0,1}, g in {0..3}, each core handling 4 of the 16 heads for one batch
element.  Each core computes a partial output projection (its heads' slice
of W_proj rows); the host sums the 4 partials per batch and adds b_proj.

Per-core schedule: the kernel is jointly limited by ScalarE (128 exp
activations over [128,1024] score tiles ~ 137us) and the PE (~137us of
matmuls), so the whole design aims at keeping the exp stream back-to-back
while PE work (qkv projection chains A/B, scores S, attention@v U, output
projection E) drains in the gaps:

  - 8 "waves", one per (query-block, head-pair), ordered
    (0,0),(1,0),(0,1),(1,1),(2,0),(3,0),(2,1),(3,1) so the A-chain and
    E-block work spreads across the exp-bound steady state.
  - psS pool (2x2 banks) score tiles; exp paces S via slot reuse.
  - psU pool (2x1 banks) U accumulators, eagerly evicted to SBUF so the
    banks free fast; softmax denominator rides as v's ones column (row 64).
  - psAB pool (2x1 banks) for A/B/E chains so they interleave anywhere.
  - v stationary padded to a 128-column window so FWL keeps LDWEIGHTS
    off the critical path in the U matmuls.
  - normalization: reciprocal on [1,512] rows, DRAM-roundtrip broadcast
    (off critical path), DVE multiplies.
  - input DMAs batched (few big transfers, sync+gpsimd queues); ACT exp
    table prewarmed by a dummy activation at t=0.
"""

import sys

if "/opt/trn_rl_repo" not in sys.path:
    sys.path.insert(0, "/opt/trn_rl_repo")

import numpy as np
import ml_dtypes

import concourse.bass as bass
import concourse.mybir as mybir
import concourse.tile as tile
from concourse.bass_utils import run_bass_kernel_spmd

DIM = 1024
N = 2048
B = 2
HEADS = 16
HEAD_DIM = 64
SCALE = HEAD_DIM ** -0.5
HPG = 4          # heads per group (per core)
GC = HPG * HEAD_DIM  # channels per core = 256
BF16 = mybir.dt.bfloat16
F32 = mybir.dt.float32

KT = DIM // 128      # 8 contraction tiles over model dim
NB = N // 512        # 4 query blocks
NKT = N // 128       # 16 key tiles
VPAD = 3 * 65 + 128  # v free size: 4 heads @ stride 65 + M=128 window pad


def _split_multi_waits(nc, max_waits=1):
    """The walrus build in this container accepts at most one sync-wait per
    instruction.  Hoist extra waits onto single-wait NOPs inserted just
    before the instruction in its engine's program order."""
    uid = [0]
    for f in nc.m.functions:
        for bb in f.blocks:
            insts = bb.instructions
            new = []
            changed = False
            for inst in insts:
                si = inst.sync_info
                if si is not None and len(si.on_wait) > max_waits:
                    waits = list(si.on_wait)
                    for w in waits[:-max_waits]:
                        nop = mybir.InstNoOp(
                            name=f"I-splitw-{uid[0]}", ins=[], outs=[])
                        uid[0] += 1
                        nop.engine = inst.engine
                        nop.sync_info = mybir.SyncInfo(
                            on_wait=[w], on_update=[])
                        new.append(nop)
                    si.on_wait = waits[-max_waits:]
                    inst.sync_info = si
                    changed = True
                new.append(inst)
            if changed:
                bb.instructions = new


def build_core_kernel() -> bass.Bass:
    nc = bass.Bass()
    xT = nc.declare_dram_parameter("xT", [DIM, N], BF16, isOutput=False)
    wqk = nc.declare_dram_parameter("wqk", [DIM, 2 * GC], BF16, isOutput=False)
    wv = nc.declare_dram_parameter("wv", [DIM, GC], BF16, isOutput=False)
    wp = nc.declare_dram_parameter("wp", [GC, DIM], BF16, isOutput=False)
    out = nc.declare_dram_parameter("out", [N, DIM], BF16, isOutput=True)

    xT_r = xT.rearrange("(kt p) n -> p kt n", p=128)
    wqk_r = wqk.rearrange("(kt p) c -> p kt c", p=128)
    wv_r = wv.rearrange("(kt p) c -> p kt c", p=128)
    wp_r = wp.rearrange("(pair p) c -> p pair c", p=128)

    with tile.TileContext(nc) as tc:
        from contextlib import ExitStack

        with ExitStack() as ctx:
            consts = ctx.enter_context(tc.tile_pool(name="consts", bufs=1))
            sbuf = ctx.enter_context(tc.tile_pool(name="sbuf", bufs=1))

            # --- resident SBUF tensors -------------------------------------
            xT_sb = sbuf.tile([128, KT, N], BF16, tag="xT")
            wqk_sb = consts.tile([128, KT, 2 * GC], BF16, tag="wqk")
            wv_sb = consts.tile([128, KT, GC], BF16, tag="wv")
            wp_sb = consts.tile([128, 2, DIM], BF16, tag="wp")
            warm_sb = consts.tile([128, 16], BF16, tag="warm")
            qk_sb = sbuf.tile([128, 4, N], BF16, tag="qk")
            # v with a ones column appended per head ([v_h | 1], stride 65):
            # the ones column turns attention@v into a matmul that also
            # emits the softmax denominator as output row 64.  Free size
            # padded so every head has a 128-col stationary window (keeps
            # FWL active -> LDWEIGHTS backgrounded).
            v_sb = sbuf.tile([128, NKT, VPAD], BF16, tag="v")
            o_sb = sbuf.tile([128, 2, N], BF16, tag="o")

            # ACT exp-table prewarm: runs at t=0, hides the ~2.7us
            # PSEUDO_LOAD_ACT_FUNC_SET under the input DMAs.
            nc.vector.memset(warm_sb[:], 1.0)
            nc.scalar.activation(
                warm_sb[:], warm_sb[:],
                mybir.ActivationFunctionType.Exp, scale=1.0)
            nc.vector.memset(v_sb[:], 1.0)

            # --- batched input DMAs ----------------------------------------
            # priority loads on the sync queue (first waves need wqk + x
            # block 0); bulk on the gpsimd queue.
            nc.sync.dma_start(out=wqk_sb[:], in_=wqk_r[:])
            nc.sync.dma_start(out=xT_sb[:, :, 0:512], in_=xT_r[:, :, 0:512])
            for nb in range(1, NB):
                nc.sync.dma_start(
                    out=xT_sb[:, :, nb * 512:(nb + 1) * 512],
                    in_=xT_r[:, :, nb * 512:(nb + 1) * 512],
                )
            nc.sync.dma_start(out=wv_sb[:], in_=wv_r[:])
            nc.sync.dma_start(out=wp_sb[:], in_=wp_r[:])

            # --- helper emitters -------------------------------------------
            def a_chain(pool, ct, nb):
                acc = pool.tile([128, 512], F32, tag="ab")
                for kt in range(KT):
                    nc.tensor.matmul(
                        acc[:],
                        lhsT=wqk_sb[:, kt, ct * 128:(ct + 1) * 128],
                        rhs=xT_sb[:, kt, nb * 512:(nb + 1) * 512],
                        start=(kt == 0),
                        stop=(kt == KT - 1),
                    )
                nc.vector.tensor_copy(
                    qk_sb[:, ct, nb * 512:(nb + 1) * 512], acc[:]
                )

            def b_chain(pool, nt):
                acc = pool.tile([128, 512], F32, tag="ab")
                for kt in range(KT):
                    nc.tensor.matmul(
                        acc[:, 0:GC],
                        lhsT=xT_sb[:, kt, nt * 128:(nt + 1) * 128],
                        rhs=wv_sb[:, kt, :],
                        start=(kt == 0),
                        stop=(kt == KT - 1),
                    )
                for h in range(HPG):
                    nc.vector.tensor_copy(
                        v_sb[:, nt, h * 65:h * 65 + 64],
                        acc[:, h * 64:(h + 1) * 64],
                    )

            def s_exp_block(psS, nqb, pair, e_t, nkts=None):
                qt = qk_sb[:, pair, :]
                kt_sb = qk_sb[:, 2 + pair, :]
                for nkt in (range(NKT) if nkts is None else nkts):
                    st = psS.tile([128, 1024], F32, tag="st")
                    for hh in range(2):
                        nc.tensor.matmul(
                            st[:, hh * 512:(hh + 1) * 512],
                            lhsT=kt_sb[
                                hh * 64:(hh + 1) * 64,
                                nkt * 128:(nkt + 1) * 128,
                            ],
                            rhs=qt[
                                hh * 64:(hh + 1) * 64,
                                nqb * 512:(nqb + 1) * 512,
                            ],
                            start=True,
                            stop=True,
                        )
                    nc.scalar.activation(
                        e_t[:, nkt, :],
                        st[:],
                        mybir.ActivationFunctionType.Exp,
                        scale=SCALE,
                    )

            def u_norm(psU, upool, rpool, rdram, rrpool, nqb, pair, e_t):
                # U_aug = [v|1]^T E^T per head with a padded 128-col
                # stationary window: rows 0-63 the head's v, row 64 the
                # softmax denominator, rows 65+ garbage (ignored).
                u_a = psU.tile([128, 512], F32, tag="u")
                u_b = psU.tile([128, 512], F32, tag="u")
                for nkt in range(NKT):
                    for hh, u_t in ((0, u_a), (1, u_b)):
                        h = pair * 2 + hh
                        nc.tensor.matmul(
                            u_t[:],
                            lhsT=v_sb[:, nkt, h * 65:h * 65 + 128],
                            rhs=e_t[:, nkt, hh * 512:(hh + 1) * 512],
                            start=(nkt == 0),
                            stop=(nkt == NKT - 1),
                        )
                # eager eviction: move the useful rows to SBUF so the PSUM
                # banks free immediately; the rest of the normalization is
                # SBUF-side and off the critical path.  SBUF operands of a
                # DVE op must share the start partition, so head b lands at
                # partitions 64-127 and the denominators at rows 0/1.
                u_sb = upool.tile([128, 512], F32, tag="usb")
                d_sb = rpool.tile([33, 512], F32, tag="d")
                nc.vector.tensor_copy(u_sb[0:64, :], u_a[0:64, :])
                nc.vector.tensor_copy(u_sb[64:128, :], u_b[0:64, :])
                # DVE SBUF accesses must start 32-aligned: denominator rows
                # at partitions 0 and 32; rows 1-31 are unused garbage.
                nc.vector.tensor_copy(d_sb[0:1, :], u_a[64:65, :])
                nc.vector.tensor_copy(d_sb[32:33, :], u_b[64:65, :])
                r_t = rpool.tile([33, 512], F32, tag="r")
                nc.vector.reciprocal(r_t[:], d_sb[:])
                # broadcast 1 -> 64 partitions via a DRAM round trip
                r_dr = rdram.tile([2, 512], F32, tag="rdr")
                rr_t = rrpool.tile([128, 512], F32, tag="rr")
                for hh in range(2):
                    nc.sync.dma_start(
                        out=r_dr[hh:hh + 1, :],
                        in_=r_t[hh * 32:hh * 32 + 1, :],
                    )
                    nc.sync.dma_start(
                        out=rr_t[hh * 64:(hh + 1) * 64, :],
                        in_=r_dr[hh:hh + 1, :].to_broadcast([64, 512]),
                    )
                nc.vector.tensor_mul(
                    o_sb[0:64, pair, nqb * 512:(nqb + 1) * 512],
                    u_sb[0:64, :],
                    rr_t[0:64, :],
                )
                nc.vector.tensor_mul(
                    o_sb[64:128, pair, nqb * 512:(nqb + 1) * 512],
                    u_sb[64:128, :],
                    rr_t[64:128, :],
                )

            def e_block(pool, opool, nqb, mts=None):
                # partial output projection for query block nqb
                for mt in (range(nqb * 4, nqb * 4 + 4) if mts is None
                           else mts):
                    ot = opool.tile([128, DIM], BF16, tag="ot")
                    for nh in range(2):
                        acc = pool.tile([128, 512], F32, tag="ab")
                        for pair in range(2):
                            nc.tensor.matmul(
                                acc[:],
                                lhsT=o_sb[:, pair, mt * 128:(mt + 1) * 128],
                                rhs=wp_sb[:, pair, nh * 512:(nh + 1) * 512],
                                start=(pair == 0),
                                stop=(pair == 1),
                            )
                        nc.vector.tensor_copy(
                            ot[:, nh * 512:(nh + 1) * 512], acc[:]
                        )
                    nc.sync.dma_start(
                        out=out[mt * 128:(mt + 1) * 128, :], in_=ot[:]
                    )

            # --- wave schedule ---------------------------------------------
            WAVES = [(0, 0), (1, 0), (0, 1), (1, 1),
                     (2, 0), (3, 0), (2, 1), (3, 1)]

            with (
                tc.tile_pool(name="psS", bufs=2, space="PSUM") as psS,
                tc.tile_pool(name="psU", bufs=2, space="PSUM") as psU,
                tc.tile_pool(name="psAB", bufs=2, space="PSUM") as psAB,
                tc.tile_pool(name="epool", bufs=3) as epool,
                tc.tile_pool(name="upool", bufs=2) as upool,
                tc.tile_pool(name="rpool", bufs=2) as rpool,
                tc.tile_pool(name="rdram", bufs=2, space="DRAM") as rdram,
                tc.tile_pool(name="rrpool", bufs=2) as rrpool,
                tc.tile_pool(name="opool", bufs=3) as opool,
            ):
                e_q = []

                def new_e():
                    e_t = epool.tile([128, NKT, 1024], BF16, tag="e")
                    e_q.append(e_t)
                    return e_t

                # prologue: k(pair0) + q(block0) chains slide under wave 0
                a_chain(psAB, 2, 0)
                a_chain(psAB, 0, 0)
                e0 = new_e()
                s_exp_block(psS, 0, 0, e0, nkts=range(0, 4))
                for nb in range(1, NB):
                    a_chain(psAB, 2, nb)
                    s_exp_block(psS, 0, 0, e0, nkts=range(nb * 4, nb * 4 + 4))
                a_chain(psAB, 0, 1)
                a_chain(psAB, 3, 0)
                a_chain(psAB, 3, 1)
                s_exp_block(psS, 1, 0, new_e())          # wave 1
                a_chain(psAB, 3, 2)
                a_chain(psAB, 3, 3)
                a_chain(psAB, 1, 0)
                for nt in range(0, 4):
                    b_chain(psAB, nt)
                s_exp_block(psS, 0, 1, new_e())          # wave 2
                b_chain(psAB, 4)
                b_chain(psAB, 5)
                a_chain(psAB, 1, 1)
                # all v chains must be emitted before u_norm(w0) — emission
                # order is program order, so a later b_chain would be a WAR
                # (U would read the memset placeholder for those key tiles).
                for nt in range(6, 16):
                    b_chain(psAB, nt)

                # filler emissions per main-loop iteration k (consume wave
                # k, emit wave k+3).  E(nqb) goes after both its pairs'
                # norms: E0 after k=2, E1 after k=3, E2 after k=6, E3 tail.
                fillers = {
                    0: lambda: None,
                    1: lambda: a_chain(psAB, 0, 2),
                    2: lambda: (a_chain(psAB, 0, 3), a_chain(psAB, 1, 2)),
                    3: lambda: (a_chain(psAB, 1, 3),
                                e_block(psAB, opool, 0)),
                    4: lambda: e_block(psAB, opool, 1),
                    5: lambda: None,
                    6: lambda: e_block(psAB, opool, 2),
                    7: lambda: e_block(psAB, opool, 3),
                }
                for k in range(8):
                    nqb, pair = WAVES[k]
                    u_norm(psU, upool, rpool, rdram, rrpool,
                           nqb, pair, e_q[k])
                    fillers[k]()
                    if k + 3 < 8:
                        nq2, p2 = WAVES[k + 3]
                        s_exp_block(psS, nq2, p2, new_e())

    _split_multi_waits(nc)
    return nc


_NC_CACHE = None


def _get_nc():
    global _NC_CACHE
    if _NC_CACHE is None:
        _NC_CACHE = build_core_kernel()
    return _NC_CACHE


def kernel(x, importance_weights, W_qkv, W_proj, b_proj, persistence_bias,
           _results_hook=None):
    x = np.asarray(x)
    W_qkv = np.asarray(W_qkv, dtype=np.float32)
    W_proj = np.asarray(W_proj, dtype=np.float32)
    b_proj = np.asarray(b_proj, dtype=np.float32)

    bf = ml_dtypes.bfloat16
    Q = W_qkv[:, 0:DIM]
    K = W_qkv[:, DIM:2 * DIM]
    V = W_qkv[:, 2 * DIM:3 * DIM]

    in_maps = []
    for core in range(8):
        b, g = divmod(core, 4)
        sl = slice(g * GC, (g + 1) * GC)
        in_maps.append({
            "xT": np.ascontiguousarray(x[b].T).astype(bf),
            "wqk": np.ascontiguousarray(
                np.concatenate([Q[:, sl], K[:, sl]], axis=1)).astype(bf),
            "wv": np.ascontiguousarray(V[:, sl]).astype(bf),
            "wp": np.ascontiguousarray(W_proj[sl, :]).astype(bf),
        })

    nc = _get_nc()
    res = run_bass_kernel_spmd(nc, in_maps, list(range(8)))
    if _results_hook is not None:
        _results_hook(res)

    out = np.zeros((B, N, DIM), dtype=np.float32)
    for core in range(8):
        b = core // 4
        out[b] += res.results[core]["out"].astype(np.float32)
    out += b_proj[None, None, :]
    return out


# revision 14
# speedup vs baseline: 1.0005x; 1.0005x over previous
"""BirthDeathAttention kernel for 8 Trainium2 NeuronCores.

Math note: in the reference, both `persistence_bias` ([1,H,1,1]) and
`importance_weights[:, None, :, None] * 0.1` ([B,1,N,1]) are constant along
the softmax (key) axis, so they cancel exactly inside the softmax.  The
module is therefore plain multi-head attention + output projection.

Sharding (per the tensor-parallel hint): core = (batch b, head-group g),
b in {0,1}, g in {0..3}, each core handling 4 of the 16 heads for one batch
element.  Each core computes a partial output projection (its heads' slice
of W_proj rows); the host sums the 4 partials per batch and adds b_proj.

Per-core schedule: the kernel is jointly limited by ScalarE (128 exp
activations over [128,1024] score tiles ~ 137us) and the PE (~137us of
matmuls), so the whole design aims at keeping the exp stream back-to-back
while PE work (qkv projection chains A/B, scores S, attention@v U, output
projection E) drains in the gaps:

  - 8 "waves", one per (query-block, head-pair), ordered
    (0,0),(1,0),(0,1),(1,1),(2,0),(3,0),(2,1),(3,1) so the A-chain and
    E-block work spreads across the exp-bound steady state.
  - psS pool (2x2 banks) score tiles; exp paces S via slot reuse.
  - psU pool (2x1 banks) U accumulators, eagerly evicted to SBUF so the
    banks free fast; softmax denominator rides as v's ones column (row 64).
  - psAB pool (2x1 banks) for A/B/E chains so they interleave anywhere.
  - v stationary padded to a 128-column window so FWL keeps LDWEIGHTS
    off the critical path in the U matmuls.
  - normalization: reciprocal on [1,512] rows, DRAM-roundtrip broadcast
    (off critical path), DVE multiplies.
  - input DMAs batched (few big transfers, sync+gpsimd queues); ACT exp
    table prewarmed by a dummy activation at t=0.
"""

import sys

if "/opt/trn_rl_repo" not in sys.path:
    sys.path.insert(0, "/opt/trn_rl_repo")

import numpy as np
import ml_dtypes

import concourse.bass as bass
import concourse.mybir as mybir
import concourse.tile as tile
from concourse.bass_utils import run_bass_kernel_spmd

DIM = 1024
N = 2048
B = 2
HEADS = 16
HEAD_DIM = 64
SCALE = HEAD_DIM ** -0.5
HPG = 4          # heads per group (per core)
GC = HPG * HEAD_DIM  # channels per core = 256
BF16 = mybir.dt.bfloat16
F32 = mybir.dt.float32

KT = DIM // 128      # 8 contraction tiles over model dim
NB = N // 512        # 4 query blocks
NKT = N // 128       # 16 key tiles
VPAD = 3 * 65 + 128  # v free size: 4 heads @ stride 65 + M=128 window pad


def _split_multi_waits(nc, max_waits=1):
    """The walrus build in this container accepts at most one sync-wait per
    instruction.  Hoist extra waits onto single-wait NOPs inserted just
    before the instruction in its engine's program order."""
    uid = [0]
    for f in nc.m.functions:
        for bb in f.blocks:
            insts = bb.instructions
            new = []
            changed = False
            for inst in insts:
                si = inst.sync_info
                if si is not None and len(si.on_wait) > max_waits:
                    waits = list(si.on_wait)
                    for w in waits[:-max_waits]:
                        nop = mybir.InstNoOp(
                            name=f"I-splitw-{uid[0]}", ins=[], outs=[])
                        uid[0] += 1
                        nop.engine = inst.engine
                        nop.sync_info = mybir.SyncInfo(
                            on_wait=[w], on_update=[])
                        new.append(nop)
                    si.on_wait = waits[-max_waits:]
                    inst.sync_info = si
                    changed = True
                new.append(inst)
            if changed:
                bb.instructions = new


def build_core_kernel() -> bass.Bass:
    nc = bass.Bass()
    xT = nc.declare_dram_parameter("xT", [DIM, N], BF16, isOutput=False)
    wqk = nc.declare_dram_parameter("wqk", [DIM, 2 * GC], BF16, isOutput=False)
    wv = nc.declare_dram_parameter("wv", [DIM, GC], BF16, isOutput=False)
    wp = nc.declare_dram_parameter("wp", [GC, DIM], BF16, isOutput=False)
    out = nc.declare_dram_parameter("out", [N, DIM], BF16, isOutput=True)

    xT_r = xT.rearrange("(kt p) n -> p kt n", p=128)
    wqk_r = wqk.rearrange("(kt p) c -> p kt c", p=128)
    wv_r = wv.rearrange("(kt p) c -> p kt c", p=128)
    wp_r = wp.rearrange("(pair p) c -> p pair c", p=128)

    with tile.TileContext(nc) as tc:
        from contextlib import ExitStack

        with ExitStack() as ctx:
            consts = ctx.enter_context(tc.tile_pool(name="consts", bufs=1))
            sbuf = ctx.enter_context(tc.tile_pool(name="sbuf", bufs=1))

            # --- resident SBUF tensors -------------------------------------
            xT_sb = sbuf.tile([128, KT, N], BF16, tag="xT")
            wqk_sb = consts.tile([128, KT, 2 * GC], BF16, tag="wqk")
            wv_sb = consts.tile([128, KT, GC], BF16, tag="wv")
            wp_sb = consts.tile([128, 2, DIM], BF16, tag="wp")
            warm_sb = consts.tile([128, 16], BF16, tag="warm")
            qk_sb = sbuf.tile([128, 4, N], BF16, tag="qk")
            # v with a ones column appended per head ([v_h | 1], stride 65):
            # the ones column turns attention@v into a matmul that also
            # emits the softmax denominator as output row 64.  Free size
            # padded so every head has a 128-col stationary window (keeps
            # FWL active -> LDWEIGHTS backgrounded).
            v_sb = sbuf.tile([128, NKT, VPAD], BF16, tag="v")
            o_sb = sbuf.tile([128, 2, N], BF16, tag="o")

            # ACT exp-table prewarm: runs at t=0, hides the ~2.7us
            # PSEUDO_LOAD_ACT_FUNC_SET under the input DMAs.
            nc.vector.memset(warm_sb[:], 1.0)
            nc.scalar.activation(
                warm_sb[:], warm_sb[:],
                mybir.ActivationFunctionType.Exp, scale=1.0)
            nc.vector.memset(v_sb[:], 1.0)

            # --- batched input DMAs ----------------------------------------
            # issue in need-order: wave 0 needs wqk's k-pair0 slice (ct=2)
            # + q-pair0 block0 slice (ct=0) + x block 0 first.  Transfers
            # parallelize across the 16 DMA engines regardless of queue.
            for ct in (2, 0, 3, 1):
                nc.sync.dma_start(
                    out=wqk_sb[:, :, ct * 128:(ct + 1) * 128],
                    in_=wqk_r[:, :, ct * 128:(ct + 1) * 128],
                )
                if ct == 2:
                    nc.sync.dma_start(
                        out=xT_sb[:, :, 0:512], in_=xT_r[:, :, 0:512])
            for nb in range(1, NB):
                nc.sync.dma_start(
                    out=xT_sb[:, :, nb * 512:(nb + 1) * 512],
                    in_=xT_r[:, :, nb * 512:(nb + 1) * 512],
                )
            nc.sync.dma_start(out=wv_sb[:], in_=wv_r[:])
            nc.sync.dma_start(out=wp_sb[:], in_=wp_r[:])

            # --- helper emitters -------------------------------------------
            def a_chain(pool, ct, nb):
                acc = pool.tile([128, 512], F32, tag="ab")
                for kt in range(KT):
                    nc.tensor.matmul(
                        acc[:],
                        lhsT=wqk_sb[:, kt, ct * 128:(ct + 1) * 128],
                        rhs=xT_sb[:, kt, nb * 512:(nb + 1) * 512],
                        start=(kt == 0),
                        stop=(kt == KT - 1),
                    )
                nc.vector.tensor_copy(
                    qk_sb[:, ct, nb * 512:(nb + 1) * 512], acc[:]
                )

            def b_chain(pool, nt):
                acc = pool.tile([128, 512], F32, tag="ab")
                for kt in range(KT):
                    nc.tensor.matmul(
                        acc[:, 0:GC],
                        lhsT=xT_sb[:, kt, nt * 128:(nt + 1) * 128],
                        rhs=wv_sb[:, kt, :],
                        start=(kt == 0),
                        stop=(kt == KT - 1),
                    )
                for h in range(HPG):
                    nc.vector.tensor_copy(
                        v_sb[:, nt, h * 65:h * 65 + 64],
                        acc[:, h * 64:(h + 1) * 64],
                    )

            def s_exp_block(psS, nqb, pair, e_t, nkts=None):
                qt = qk_sb[:, pair, :]
                kt_sb = qk_sb[:, 2 + pair, :]
                for nkt in (range(NKT) if nkts is None else nkts):
                    st = psS.tile([128, 1024], F32, tag="st")
                    for hh in range(2):
                        nc.tensor.matmul(
                            st[:, hh * 512:(hh + 1) * 512],
                            lhsT=kt_sb[
                                hh * 64:(hh + 1) * 64,
                                nkt * 128:(nkt + 1) * 128,
                            ],
                            rhs=qt[
                                hh * 64:(hh + 1) * 64,
                                nqb * 512:(nqb + 1) * 512,
                            ],
                            start=True,
                            stop=True,
                        )
                    nc.scalar.activation(
                        e_t[:, nkt, :],
                        st[:],
                        mybir.ActivationFunctionType.Exp,
                        scale=SCALE,
                    )

            def u_quarter(u_a, u_b, pair, e_t, nkts):
                # U_aug = [v|1]^T E^T per head with a padded 128-col
                # stationary window: rows 0-63 the head's v, row 64 the
                # softmax denominator, rows 65+ garbage (ignored).
                for nkt in nkts:
                    for hh, u_t in ((0, u_a), (1, u_b)):
                        h = pair * 2 + hh
                        nc.tensor.matmul(
                            u_t[:],
                            lhsT=v_sb[:, nkt, h * 65:h * 65 + 128],
                            rhs=e_t[:, nkt, hh * 512:(hh + 1) * 512],
                            start=(nkt == 0),
                            stop=(nkt == NKT - 1),
                        )

            def norm_finish(u_a, u_b, upool, rpool, rdram, rrpool,
                            nqb, pair):
                # eager eviction: move the useful rows to SBUF so the PSUM
                # banks free immediately; the rest of the normalization is
                # SBUF-side and off the critical path.  SBUF operands of a
                # DVE op must share the start partition, so head b lands at
                # partitions 64-127 and the denominators at rows 0/1.
                u_sb = upool.tile([128, 512], F32, tag="usb")
                d_sb = rpool.tile([33, 512], F32, tag="d")
                nc.vector.tensor_copy(u_sb[0:64, :], u_a[0:64, :])
                nc.vector.tensor_copy(u_sb[64:128, :], u_b[0:64, :])
                # DVE SBUF accesses must start 32-aligned: denominator rows
                # at partitions 0 and 32; rows 1-31 are unused garbage.
                nc.vector.tensor_copy(d_sb[0:1, :], u_a[64:65, :])
                nc.vector.tensor_copy(d_sb[32:33, :], u_b[64:65, :])
                r_t = rpool.tile([33, 512], F32, tag="r")
                nc.vector.reciprocal(r_t[:], d_sb[:])
                # broadcast 1 -> 64 partitions via a DRAM round trip
                r_dr = rdram.tile([2, 512], F32, tag="rdr")
                rr_t = rrpool.tile([128, 512], F32, tag="rr")
                for hh in range(2):
                    nc.sync.dma_start(
                        out=r_dr[hh:hh + 1, :],
                        in_=r_t[hh * 32:hh * 32 + 1, :],
                    )
                    nc.sync.dma_start(
                        out=rr_t[hh * 64:(hh + 1) * 64, :],
                        in_=r_dr[hh:hh + 1, :].to_broadcast([64, 512]),
                    )
                nc.vector.tensor_mul(
                    o_sb[0:64, pair, nqb * 512:(nqb + 1) * 512],
                    u_sb[0:64, :],
                    rr_t[0:64, :],
                )
                nc.vector.tensor_mul(
                    o_sb[64:128, pair, nqb * 512:(nqb + 1) * 512],
                    u_sb[64:128, :],
                    rr_t[64:128, :],
                )

            def e_block(pool, opool, nqb, mts=None):
                # partial output projection for query block nqb
                for mt in (range(nqb * 4, nqb * 4 + 4) if mts is None
                           else mts):
                    ot = opool.tile([128, DIM], BF16, tag="ot")
                    for nh in range(2):
                        acc = pool.tile([128, 512], F32, tag="ab")
                        for pair in range(2):
                            nc.tensor.matmul(
                                acc[:],
                                lhsT=o_sb[:, pair, mt * 128:(mt + 1) * 128],
                                rhs=wp_sb[:, pair, nh * 512:(nh + 1) * 512],
                                start=(pair == 0),
                                stop=(pair == 1),
                            )
                        nc.vector.tensor_copy(
                            ot[:, nh * 512:(nh + 1) * 512], acc[:]
                        )
                    nc.sync.dma_start(
                        out=out[mt * 128:(mt + 1) * 128, :], in_=ot[:]
                    )

            # --- wave schedule ---------------------------------------------
            WAVES = [(0, 0), (1, 0), (0, 1), (1, 1),
                     (2, 0), (3, 0), (2, 1), (3, 1)]

            with (
                tc.tile_pool(name="psS", bufs=2, space="PSUM") as psS,
                tc.tile_pool(name="psU", bufs=2, space="PSUM") as psU,
                tc.tile_pool(name="psAB", bufs=2, space="PSUM") as psAB,
                tc.tile_pool(name="epool", bufs=3) as epool,
                tc.tile_pool(name="upool", bufs=2) as upool,
                tc.tile_pool(name="rpool", bufs=2) as rpool,
                tc.tile_pool(name="rdram", bufs=2, space="DRAM") as rdram,
                tc.tile_pool(name="rrpool", bufs=2) as rrpool,
                tc.tile_pool(name="opool", bufs=3) as opool,
            ):
                e_q = []

                def new_e():
                    e_t = epool.tile([128, NKT, 1024], BF16, tag="e")
                    e_q.append(e_t)
                    return e_t

                # prologue: k(pair0) + q(block0) chains slide under wave 0
                a_chain(psAB, 2, 0)
                a_chain(psAB, 0, 0)
                e0 = new_e()
                s_exp_block(psS, 0, 0, e0, nkts=range(0, 4))
                for nb in range(1, NB):
                    a_chain(psAB, 2, nb)
                    s_exp_block(psS, 0, 0, e0, nkts=range(nb * 4, nb * 4 + 4))
                a_chain(psAB, 0, 1)
                a_chain(psAB, 3, 0)
                a_chain(psAB, 3, 1)
                s_exp_block(psS, 1, 0, new_e())          # wave 1
                a_chain(psAB, 3, 2)
                a_chain(psAB, 3, 3)
                a_chain(psAB, 1, 0)
                for nt in range(0, 4):
                    b_chain(psAB, nt)
                # wave 2 quarters interleaved with the remaining v chains
                # (all v chains must be emitted before u(w0) — emission
                # order is program order, a later b_chain would be a WAR
                # and U would read the memset placeholder).
                e2_ = new_e()
                bq = [[4, 5], [6, 7, 8], [9, 10, 11], [12, 13, 14, 15]]
                for q in range(4):
                    s_exp_block(psS, 0, 1, e2_, nkts=range(q * 4, q * 4 + 4))
                    for nt in bq[q]:
                        b_chain(psAB, nt)
                    if q == 0:
                        a_chain(psAB, 1, 1)

                # per-iteration fillers, spread per quarter q.  E(nqb) goes
                # after both its pairs' norms: E0 after k=2, E1 after k=3,
                # E2 after k=6, E3 after k=7.
                fillers = {
                    (1, 0): lambda: a_chain(psAB, 0, 2),
                    (2, 0): lambda: a_chain(psAB, 0, 3),
                    (2, 2): lambda: a_chain(psAB, 1, 2),
                    (3, 0): lambda: a_chain(psAB, 1, 3),
                    (3, 1): lambda: e_block(psAB, opool, 0, mts=range(0, 2)),
                    (3, 3): lambda: e_block(psAB, opool, 0, mts=range(2, 4)),
                    (4, 0): lambda: e_block(psAB, opool, 1, mts=range(4, 5)),
                    (4, 1): lambda: e_block(psAB, opool, 1, mts=range(5, 6)),
                    (4, 2): lambda: e_block(psAB, opool, 1, mts=range(6, 7)),
                    (4, 3): lambda: e_block(psAB, opool, 1, mts=range(7, 8)),
                    (7, 0): lambda: e_block(psAB, opool, 2, mts=range(8, 10)),
                    (7, 2): lambda: e_block(psAB, opool, 2,
                                            mts=range(10, 12)),
                }
                for k in range(8):
                    nqb, pair = WAVES[k]
                    u_a = psU.tile([128, 512], F32, tag="u")
                    u_b = psU.tile([128, 512], F32, tag="u")
                    if k + 3 < 8:
                        e_next = new_e()
                        nq2, p2 = WAVES[k + 3]
                    for q in range(4):
                        u_quarter(u_a, u_b, pair, e_q[k],
                                  range(q * 4, q * 4 + 4))
                        f = fillers.get((k, q))
                        if f is not None:
                            f()
                        if k + 3 < 8:
                            s_exp_block(psS, nq2, p2, e_next,
                                        nkts=range(q * 4, q * 4 + 4))
                    norm_finish(u_a, u_b, upool, rpool, rdram, rrpool,
                                nqb, pair)
                e_block(psAB, opool, 3)

    _split_multi_waits(nc)
    return nc


_NC_CACHE = None


def _get_nc():
    global _NC_CACHE
    if _NC_CACHE is None:
        _NC_CACHE = build_core_kernel()
    return _NC_CACHE


def kernel(x, importance_weights, W_qkv, W_proj, b_proj, persistence_bias,
           _results_hook=None):
    x = np.asarray(x)
    W_qkv = np.asarray(W_qkv, dtype=np.float32)
    W_proj = np.asarray(W_proj, dtype=np.float32)
    b_proj = np.asarray(b_proj, dtype=np.float32)

    bf = ml_dtypes.bfloat16
    Q = W_qkv[:, 0:DIM]
    K = W_qkv[:, DIM:2 * DIM]
    V = W_qkv[:, 2 * DIM:3 * DIM]

    in_maps = []
    for core in range(8):
        b, g = divmod(core, 4)
        sl = slice(g * GC, (g + 1) * GC)
        in_maps.append({
            "xT": np.ascontiguousarray(x[b].T).astype(bf),
            "wqk": np.ascontiguousarray(
                np.concatenate([Q[:, sl], K[:, sl]], axis=1)).astype(bf),
            "wv": np.ascontiguousarray(V[:, sl]).astype(bf),
            "wp": np.ascontiguousarray(W_proj[sl, :]).astype(bf),
        })

    nc = _get_nc()
    res = run_bass_kernel_spmd(nc, in_maps, list(range(8)))
    if _results_hook is not None:
        _results_hook(res)

    out = np.zeros((B, N, DIM), dtype=np.float32)
    for core in range(8):
        b = core // 4
        out[b] += res.results[core]["out"].astype(np.float32)
    out += b_proj[None, None, :]
    return out
